# revision 1
# baseline (speedup 1.0000x reference)
"""CLD sde_reverse (Riemann geometry) Trainium2 kernel.

Contract: kernel(u, score_x, t) -> (drift, diffusion), full (unsharded) numpy
arrays, computed on 8 NeuronCores via bass/Tile + run_bass_kernel_spmd.

Sharding: pixels (image rows) are sharded 8 ways; every core sees all 64 batch
elements for its 32 rows. The batch-mean outer product G, the 3x3
inverse/cholesky, and the drift matmuls are all per-pixel, so there are no
cross-core dependencies and no collectives.

Math (per pixel, 3x3):
    G     = alpha * (mean_b s s^T)/norm + (1-alpha)/m_inv * I
    L     = chol(G),  Ginv = adj(G)/det(G)
    A     = beta * L @ Ginv
    drift_x = A @ r
    drift_r = -(beta*L) @ x - beta*Gamma * G @ (Ginv @ r)
            = -(beta*L) @ x - beta*Gamma * r          (G @ Ginv = I exactly)
    diffusion_x = 0
    diffusion_r = sqrt(2*beta*Gamma) * (L @ 1)        (batch independent)

Device layout per core: pixel p in [0,8192) maps to (part, pl) = (p>>6, p&63);
tensors are [channel, 128 part, 64 batch, 64 pl] so every DMA run is
contiguous.  G & coefficients are fp32; the big batched elementwise stage runs
in fp16 (rel err ~1e-3 vs the fp32 reference).
"""

import math

import numpy as np

# ---- model constants (from the reference config) ----
M_INV = 4.0
GAMMA_BIG = 0.04
BETA0 = 4.0
RIEMANN_MIX = 0.5
K_DECAY = 4.5
C = 3
HW = 256
B = 64

N_CORES = 8
ROWS = HW // N_CORES  # 32 rows per core
P = 128               # SBUF partitions
PL = (ROWS * HW) // P  # 64 free pixels per partition

BETA_C = BETA0 * math.sqrt(M_INV)        # 8.0
GAMMA_C = GAMMA_BIG * math.sqrt(M_INV)   # 0.08
BG = BETA_C * GAMMA_C                    # 0.64
BG_SCALE = math.sqrt(2.0 * BETA_C * GAMMA_C)

_PROG_CACHE: dict = {}


def _build_program(ca: float, cid: float, main_fp16: bool = True,
                   n_reps: int = 1, g_via_act: bool = True):
    """Build + compile the per-core SPMD bass program.

    ca  = alpha / (B * normalization)   (scale for the raw sum S_ij)
    cid = (1 - alpha) / M_INV           (identity mixture term)
    """
    from contextlib import ExitStack

    import concourse.bacc as bacc
    import concourse.mybir as mybir
    import concourse.tile as tile

    dt = mybir.dt
    op = mybir.AluOpType
    f32 = dt.float32
    f16 = dt.float16 if main_fp16 else dt.float32
    AF = mybir.ActivationFunctionType

    nc = bacc.Bacc("TRN2", target_bir_lowering=False, debug=False,
                   num_devices=N_CORES)

    sdt = f16
    s_in = nc.dram_tensor("s_in", [C, P, B, PL], sdt,
                          kind="ExternalInput").ap()
    u_in = nc.dram_tensor("u_in", [2 * C, P, B, PL], f16,
                          kind="ExternalInput").ap()
    id_in = nc.dram_tensor("ident", [P, P], dt.float16,
                           kind="ExternalInput").ap()
    drift_o = nc.dram_tensor("drift", [2 * C, P, B, PL], f16,
                             kind="ExternalOutput").ap()
    dif_o = nc.dram_tensor("dif", [C, P, PL], f32, kind="ExternalOutput").ap()

    HB = B // 2   # batch half
    QB = B // 4   # batch quarter

    with tile.TileContext(nc) as tc:
      for _rep in range(n_reps):
        with ExitStack() as stack:
            coef = stack.enter_context(tc.tile_pool(name="coef", bufs=1))
            data = stack.enter_context(tc.tile_pool(name="data", bufs=1))
            tmp = stack.enter_context(tc.tile_pool(name="tmp", bufs=2))
            ident = coef.tile([P, P], f16, tag="ident")

            # ------------- stage A: G_ij = ca * sum_b s_i s_j + cid*I ------
            g = {}
            with tc.tile_pool(name="score", bufs=1) as score_pool, \
                 tc.tile_pool(name="prod", bufs=2) as prod_pool:
                s_t = [score_pool.tile([P, B, PL], sdt, tag=f"s{c}",
                                        name=f"s{c}") for c in range(C)]
                # channel-major half order: the first cross product needs
                # (s0,s1) halves, so land those first
                for bh in range(2):
                    bsl = slice(bh * HB, (bh + 1) * HB)
                    for c in range(C):
                        nc.sync.dma_start(out=s_t[c][:, bsl, :],
                                          in_=s_in[c, :, bsl, :])
                # pre-load u (overlaps with the G/coefficient stages)
                u_t = []
                for c in range(2 * C):
                    ut = data.tile([P, B, PL], f16, tag=f"u{c}")
                    for bh in range(2):
                        bsl = slice(bh * HB, (bh + 1) * HB)
                        nc.sync.dma_start(out=ut[:, bsl, :],
                                          in_=u_in[c, :, bsl, :])
                    u_t.append(ut)
                x_t, r_t = u_t[:C], u_t[C:]
                nc.sync.dma_start(out=ident[:], in_=id_in[:])

                R = {}

                def sq_reduce(src_ap, key, square=True):
                    # (optionally) square on ACT (fp16 out), then a 2-level
                    # fp16 batch fold (64->16) before the 1x tensor_reduce.
                    if square:
                        sq = prod_pool.tile([P, B, PL], f16, tag="sq")
                        nc.scalar.activation(sq[:], src_ap, AF.Square)
                    else:
                        sq = src_ap.tensor
                    f1 = prod_pool.tile([P, HB, PL], f16, tag="fold1")
                    nc.vector.tensor_tensor(
                        f1[:], sq[:, 0:HB, :], sq[:, HB:B, :], op.add)
                    f2 = prod_pool.tile([P, QB, PL], f16, tag="fold2")
                    nc.vector.tensor_tensor(
                        f2[:], f1[:, 0:QB, :], f1[:, QB:HB, :], op.add)
                    rt_ = tmp.tile([P, PL], f32, tag=f"R{key[0]}{key[1]}")
                    nc.vector.tensor_reduce(
                        rt_[:], f2[:].rearrange("p b l -> p l b"),
                        axis=mybir.AxisListType.X, op=op.add)
                    R[key] = rt_

                # cross terms: direct fp16 products on DVE (b-halved so the
                # first product starts as soon as the first half-DMAs land);
                # diagonal terms: squares on the scalar engine.
                for (i, j) in [(0, 1), (0, 2), (1, 2)]:
                    q = prod_pool.tile([P, B, PL], f16, tag="qsum")
                    for bh in range(2):
                        bsl = slice(bh * HB, (bh + 1) * HB)
                        nc.vector.tensor_tensor(
                            q[:, bsl, :], s_t[i][:, bsl, :],
                            s_t[j][:, bsl, :], op.mult)
                    sq_reduce(q[:], (i, j), square=False)
                for i in range(C):
                    sq_reduce(s_t[i][:], (i, i))
                for i in range(C):
                    gii = coef.tile([P, PL], f32, tag=f"g{i}{i}")
                    nc.scalar.activation(gii[:], R[(i, i)][:], AF.Copy,
                                         bias=float(cid), scale=float(ca))
                    g[(i, i)] = gii
                for (i, j) in [(0, 1), (0, 2), (1, 2)]:
                    gij = coef.tile([P, PL], f32, tag=f"g{i}{j}")
                    nc.scalar.mul(gij[:], R[(i, j)][:], float(ca))
                    g[(i, j)] = gij
                    g[(j, i)] = gij

            # ------------- stage B: per-pixel 3x3 coefficients -------------
            def tt(a, b_, o, tag):
                t = coef.tile([P, PL], f32, tag=tag)
                nc.vector.tensor_tensor(t[:], a[:], b_[:], o)
                return t

            def fmsub(a, b_, c_, d_, tag):
                # a*b - c*d
                t1 = tmp.tile([P, PL], f32, tag="fm1")
                nc.vector.tensor_tensor(t1[:], a[:], b_[:], op.mult)
                t2 = tmp.tile([P, PL], f32, tag="fm2")
                nc.vector.tensor_tensor(t2[:], c_[:], d_[:], op.mult)
                t = coef.tile([P, PL], f32, tag=tag)
                nc.vector.tensor_tensor(t[:], t1[:], t2[:], op.subtract)
                return t

            def to16(plane, tag):
                e = coef.tile([P, 1, PL], f16, tag=tag)
                nc.scalar.copy(e[:, 0, :], plane[:])
                return e

            # cholesky first (one Newton step on the ACT sqrt) -- the
            # drift_r path depends only on L, so its big batched ops can
            # start while the adjugate/inverse/A path is still running.
            def sqrt_ref(a, tag):
                s0 = tmp.tile([P, PL], f32, tag="sq0")
                nc.scalar.activation(s0[:], a[:], AF.Sqrt)
                r0 = tmp.tile([P, PL], f32, tag="sqr")
                nc.vector.reciprocal(r0[:], s0[:])
                ar = tmp.tile([P, PL], f32, tag="sqar")
                nc.vector.tensor_tensor(ar[:], a[:], r0[:], op.mult)
                ssum = tmp.tile([P, PL], f32, tag="sqsum")
                nc.vector.tensor_tensor(ssum[:], s0[:], ar[:], op.add)
                out = coef.tile([P, PL], f32, tag=tag)
                nc.scalar.mul(out[:], ssum[:], 0.5)
                return out

            l00 = sqrt_ref(g[0, 0], "l00")
            il00 = coef.tile([P, PL], f32, tag="il00")
            nc.vector.reciprocal(il00[:], l00[:])
            l10 = tt(g[0, 1], il00, op.mult, "l10")
            l20 = tt(g[0, 2], il00, op.mult, "l20")
            t = tt(l10, l10, op.mult, "l10sq")
            dd1 = tt(g[1, 1], t, op.subtract, "dd1")
            l11 = sqrt_ref(dd1, "l11")
            il11 = coef.tile([P, PL], f32, tag="il11")
            nc.vector.reciprocal(il11[:], l11[:])
            t = tt(l20, l10, op.mult, "l20l10")
            t = tt(g[1, 2], t, op.subtract, "g12m")
            l21 = tt(t, il11, op.mult, "l21")
            t = tt(l20, l20, op.mult, "l20sq")
            dd2 = tt(g[2, 2], t, op.subtract, "dd2a")
            t = tt(l21, l21, op.mult, "l21sq")
            dd2 = tt(dd2, t, op.subtract, "dd2")
            l22 = sqrt_ref(dd2, "l22")

            # bL = beta * L  (scaled once, reused by drift_r, A, diffusion)
            L = {}
            for (i, j), lt in [((0, 0), l00), ((1, 0), l10), ((1, 1), l11),
                               ((2, 0), l20), ((2, 1), l21), ((2, 2), l22)]:
                blt = coef.tile([P, PL], f32, tag=f"bl{i}{j}")
                nc.scalar.mul(blt[:], lt[:], BETA_C)
                L[(i, j)] = blt
            eL = {(i, j): to16(L[(i, j)], f"eL{i}{j}")[:]
                  for (i, j) in [(0, 0), (1, 0), (1, 1),
                                 (2, 0), (2, 1), (2, 2)]}

            # diffusion_r rows (batch-independent): bg/beta * row sums of bL
            bgob = BG_SCALE / BETA_C
            dif0 = coef.tile([P, PL], f32, tag="dif0")
            nc.scalar.mul(dif0[:], L[0, 0][:], bgob)
            t = tt(L[1, 0], L[1, 1], op.add, "difs1")
            dif1 = coef.tile([P, PL], f32, tag="dif1")
            nc.scalar.mul(dif1[:], t[:], bgob)
            t = tt(L[2, 0], L[2, 1], op.add, "difs2a")
            t = tt(t, L[2, 2], op.add, "difs2")
            dif2 = coef.tile([P, PL], f32, tag="dif2")
            nc.scalar.mul(dif2[:], t[:], bgob)
            for i, dtile in enumerate((dif0, dif1, dif2)):
                nc.sync.dma_start(out=dif_o[i], in_=dtile[:])

            # adjugate (symmetric): c00 = g11*g22 - g12^2, ...
            c00 = fmsub(g[1, 1], g[2, 2], g[1, 2], g[1, 2], "c00")
            c01 = fmsub(g[0, 2], g[1, 2], g[0, 1], g[2, 2], "c01")
            c02 = fmsub(g[0, 1], g[1, 2], g[0, 2], g[1, 1], "c02")
            c11 = fmsub(g[0, 0], g[2, 2], g[0, 2], g[0, 2], "c11")
            c12 = fmsub(g[0, 1], g[0, 2], g[0, 0], g[1, 2], "c12")
            c22 = fmsub(g[0, 0], g[1, 1], g[0, 1], g[0, 1], "c22")

            # det = g00*c00 + g01*c01 + g02*c02
            d0 = tt(g[0, 0], c00, op.mult, "d0")
            d1 = tt(g[0, 1], c01, op.mult, "d1")
            d2 = tt(g[0, 2], c02, op.mult, "d2")
            det = tt(d0, d1, op.add, "deta")
            det = tt(det, d2, op.add, "det")
            rdet = coef.tile([P, PL], f32, tag="rdet")
            nc.vector.reciprocal(rdet[:], det[:])

            # Ginv rows stacked as [P, 3(j), PL] tiles so each A-row is
            # computed in one broadcast op instead of 9 plane chains.
            IV = [coef.tile([P, 3, PL], f32, tag=f"IV{k}", name=f"IV{k}")
                  for k in range(3)]
            for (i, j), cof in [((0, 0), c00), ((0, 1), c01), ((0, 2), c02),
                                ((1, 1), c11), ((1, 2), c12), ((2, 2), c22)]:
                nc.vector.tensor_tensor(IV[i][:, j, :], cof[:], rdet[:],
                                        op.mult)
                if i != j:
                    nc.scalar.copy(IV[j][:, i, :], IV[i][:, j, :])

            # A-row i = sum_{k<=i} bL_ik (broadcast over j) * IV_k
            def blb(i, k):
                return L[(i, k)][:].rearrange(
                    "p l -> p () l").broadcast_to([P, 3, PL])

            AR = []
            for i in range(3):
                ar = coef.tile([P, 3, PL], f32, tag=f"AR{i}", name=f"AR{i}")
                if i == 0:
                    nc.vector.tensor_tensor(ar[:], IV[0][:], blb(0, 0),
                                            op.mult)
                else:
                    acc = tmp.tile([P, 3, PL], f32, tag="Aacc")
                    nc.vector.tensor_tensor(acc[:], IV[0][:], blb(i, 0),
                                            op.mult)
                    for k in range(1, i + 1):
                        pr = tmp.tile([P, 3, PL], f32, tag="Apr")
                        nc.vector.tensor_tensor(pr[:], IV[k][:], blb(i, k),
                                                op.mult)
                        dst = ar if k == i else tmp.tile([P, 3, PL], f32,
                                                         tag="Aacc")
                        nc.vector.tensor_tensor(dst[:], acc[:], pr[:], op.add)
                        acc = dst
                AR.append(ar)
            eAR = []
            for i in range(3):
                e = coef.tile([P, 3, PL], f16, tag=f"eAR{i}", name=f"eAR{i}")
                nc.scalar.copy(e[:], AR[i][:])
                eAR.append(e)
            eA = {(i, j): eAR[i][:, j:j + 1, :]
                  for i in range(3) for j in range(3)}

            mtmp = stack.enter_context(tc.tile_pool(name="mtmp", bufs=2))
            outs = stack.enter_context(tc.tile_pool(name="outs", bufs=2))
            psum = stack.enter_context(
                tc.tile_pool(name="psum", bufs=2, space="PSUM"))

            # ------------- stage C: batched elementwise main stage ---------
            # Coefficients are read through broadcast APs (batch dim step-0);
            # per-term products on DVE (fp16 2x); multi-term sums accumulate
            # on the tensor engine via identity matmuls into PSUM; ScalarE
            # drains PSUM back to fp16.  Work is split into batch-halves so
            # output DMAs stream while compute continues.
            def madd_psum(coeffs, ins, bh, scalar_term=None):
                n = len(coeffs) + (1 if scalar_term is not None else 0)
                bsl = slice(bh * HB, (bh + 1) * HB)
                prs = []
                for idx, (cc, dd) in enumerate(zip(coeffs, ins)):
                    pr = mtmp.tile([P, HB, PL], f16, tag=f"pr{idx}", bufs=3)
                    bc = cc.broadcast_to([P, HB, PL])
                    nc.vector.tensor_tensor(pr[:], dd[:, bsl, :], bc, op.mult)
                    prs.append(pr)
                if scalar_term is not None:
                    s_in_, s_val = scalar_term
                    pr = mtmp.tile([P, HB, PL], f16, tag="prs", bufs=3)
                    nc.vector.tensor_scalar(pr[:], s_in_[:, bsl, :], s_val,
                                            None, op.mult)
                    prs.append(pr)
                pss = []
                for bq in range(2):
                    ps = psum.tile([P, 1024], f32, tag="ps", bufs=4)
                    for s2 in range(2):
                        sl = slice(s2 * 512, (s2 + 1) * 512)
                        gl = slice(bq * 1024 + s2 * 512,
                                   bq * 1024 + (s2 + 1) * 512)
                        for idx, pr in enumerate(prs):
                            rhs = pr[:].rearrange("p b l -> p (b l)")
                            nc.tensor.matmul(
                                ps[:, sl], ident[:], rhs[:, gl],
                                start=(idx == 0), stop=(idx == n - 1))
                    pss.append(ps)
                return pss

            def drain(pss, out_tile, scale=1.0):
                for bq, ps in enumerate(pss):
                    nc.scalar.mul(
                        out_tile[:, bq * QB:(bq + 1) * QB, :],
                        ps[:].rearrange("p (b l) -> p b l", b=QB), scale)

            def emit_dx(i, bh):
                bsl = slice(bh * HB, (bh + 1) * HB)
                dx = outs.tile([P, HB, PL], f16, tag=f"dx{i}", name=f"dx{i}")
                pss = madd_psum([eA[(i, 0)], eA[(i, 1)], eA[(i, 2)]], r_t, bh)
                drain(pss, dx)
                nc.sync.dma_start(out=drift_o[i, :, bsl, :], in_=dx[:])

            def emit_dr(i, bh):
                bsl = slice(bh * HB, (bh + 1) * HB)
                dr = outs.tile([P, HB, PL], f16, tag=f"dr{i}", name=f"dr{i}")
                if i == 0:
                    # dr0 = -(bL00*x0 + BG*r0): no PE pipeline -> short tail
                    m = mtmp.tile([P, HB, PL], f16, tag="m_i", name="m")
                    bc = eL[(0, 0)].broadcast_to([P, HB, PL])
                    nc.vector.tensor_tensor(m[:], x_t[0][:, bsl, :], bc,
                                            op.mult)
                    negr = mtmp.tile([P, HB, PL], f16, tag="negr",
                                     name="negr")
                    nc.vector.tensor_scalar(negr[:], r_t[i][:, bsl, :], -BG,
                                            None, op.mult)
                    nc.vector.tensor_tensor(dr[:], negr[:], m[:], op.subtract)
                else:
                    pss = madd_psum([eL[(i, j)] for j in range(i + 1)],
                                    x_t[:i + 1], bh)
                    m = mtmp.tile([P, HB, PL], f16, tag="m_i", name="m")
                    drain(pss, m)
                    negr = mtmp.tile([P, HB, PL], f16, tag="negr",
                                     name="negr")
                    nc.vector.tensor_scalar(negr[:], r_t[i][:, bsl, :], -BG,
                                            None, op.mult)
                    nc.vector.tensor_tensor(dr[:], negr[:], m[:], op.subtract)
                nc.sync.dma_start(out=drift_o[C + i, :, bsl, :], in_=dr[:])

            # dr0 last: its chain needs no PE/ACT pipeline, so the kernel
            # tail is just one subtract + one streaming DMA.
            for bh in range(2):
                for i in range(3):
                    emit_dx(i, bh)
                emit_dr(2, bh)
                emit_dr(1, bh)
            emit_dr(0, 0)
            emit_dr(0, 1)

    nc.compile()
    return nc


def _host_reference(u, score_x, t):
    """Pure-numpy fallback (exact reference math); used only when t[0]==1.0
    (the stateful normalization branch, never hit with uniform t)."""
    x, r = u[:, :C], u[:, C:]
    s = np.transpose(score_x, (0, 2, 3, 1)).astype(np.float32)
    G = np.einsum("bhwi,bhwj->hwij", s, s) / np.float32(score_x.shape[0])
    t0 = t[0]
    diag_mean = np.mean(np.trace(G, axis1=-2, axis2=-1)) / C
    normalization = np.where(t0 == 1.0, diag_mean * M_INV, 1.0)
    G = G / normalization
    G_id = (1.0 / M_INV) * np.eye(C, dtype=G.dtype)
    alpha = RIEMANN_MIX * np.exp(-K_DECAY * (1.0 - t0))
    G = alpha * G + (1.0 - alpha) * G_id
    G_inv = np.linalg.inv(G).astype(np.float32)
    G_sqrt = np.linalg.cholesky(G).astype(np.float32)

    def mm(Amat, Bf):
        return np.einsum("hwij,bjhw->bihw", Amat, Bf).astype(np.float32)

    hr = mm(G_inv, r)
    drift_x = BETA_C * mm(G_sqrt, hr)
    drift_r = (-BETA_C * mm(G_sqrt, x) - BETA_C * GAMMA_C * mm(G, hr))
    diffusion_x = np.zeros_like(x)
    diffusion_r = BG_SCALE * mm(G_sqrt, np.ones_like(r))
    drift = np.concatenate((drift_x, drift_r), axis=1)
    diffusion = np.concatenate((diffusion_x, diffusion_r), axis=1)
    return drift, diffusion


def kernel(u, score_x, t):
    from concourse.bass_utils import run_bass_kernel_spmd

    u = np.ascontiguousarray(np.asarray(u, dtype=np.float32))
    score_x = np.ascontiguousarray(np.asarray(score_x, dtype=np.float32))
    t = np.asarray(t, dtype=np.float32)

    t0 = float(t[0])
    if t0 == 1.0:
        return _host_reference(u, score_x, t)

    alpha = RIEMANN_MIX * math.exp(-K_DECAY * (1.0 - t0))
    ca = alpha / B          # normalization == 1.0 on this branch
    cid = (1.0 - alpha) / M_INV

    key = (round(ca, 12), round(cid, 12))
    nc = _PROG_CACHE.get(key)
    if nc is None:
        nc = _build_program(ca, cid)
        _PROG_CACHE[key] = nc

    in_maps = []
    for k in range(N_CORES):
        rows = slice(k * ROWS, (k + 1) * ROWS)
        s_np = (score_x[:, :, rows, :]
                .reshape(B, C, P, PL).transpose(1, 2, 0, 3)
                .astype(np.float16))
        u_np = (u[:, :, rows, :]
                .reshape(B, 2 * C, P, PL).transpose(1, 2, 0, 3)
                .astype(np.float16))
        in_maps.append({
            "s_in": np.ascontiguousarray(s_np),
            "u_in": np.ascontiguousarray(u_np),
            "ident": np.eye(P, dtype=np.float16),
        })

    res = run_bass_kernel_spmd(nc, in_maps, list(range(N_CORES)))

    drift = np.empty((B, 2 * C, HW, HW), dtype=np.float32)
    diffusion = np.zeros((B, 2 * C, HW, HW), dtype=np.float32)
    for k in range(N_CORES):
        rows = slice(k * ROWS, (k + 1) * ROWS)
        dk = res.results[k]["drift"].astype(np.float32)     # [6, P, B, PL]
        drift[:, :, rows, :] = dk.transpose(2, 0, 1, 3).reshape(
            B, 2 * C, ROWS, HW)
        difk = res.results[k]["dif"].reshape(C, ROWS, HW)   # [3, P, PL]
        diffusion[:, C:, rows, :] = difk[None, :, :, :]
    return drift, diffusion

